# revision 1
# baseline (speedup 1.0000x reference)
"""Trainium2 Bass kernel for a linear state-space scan.

Reference computation (per batch row b):
    x_{t+1} = A x_t + B u_t          (x_0 = 0)
    out[t]  = C x_{t+1} + D u_t  =  E x_t + F u_t
with E = C A, F = C B + D.

Shapes: u [32, 4096, 128]; A, B, C, D [128, 128]; out [32, 4096, 128].

Strategy
--------
Data-parallel over batch: 32 rows / 8 cores = 4 rows per core; A/B/C/D-derived
weights replicated. No collectives.

Per core, time is split into N1 chunks of length L (L*N1 = 4096). Because
A = 0.9 * orthogonal, ||A^m|| = 0.9^m exactly, so chunk-start states are a
*short* truncated convolution over previous chunk contributions (J terms,
error ~0.9^(L*J)), making the whole computation chunk-parallel:

  1. Local scan (zero-init per chunk), all chunks in parallel as matmul
     columns:  w_{c,j+1} = A w_{c,j} + B u_{c,j}  -> L sequential matmuls
     over [128, N1*4] column slabs.  The slab is split into 512-column
     chunks forming independent scan chains, interleaved on PE so the
     PSUM->SBUF copy of one chain hides under the matmuls of others.
  2. Chunk-start states: s_c = sum_{m<J} (A^L)^m R_{c-1-m},  R_c = w_{c,L}.
  3. Outputs: out[c,j] = E w_{c,j} + F u_{c,j} + (E A^j) s_c.

Everything on-chip is kept transposed ([d=128 partitions, columns]); the
host does the (cheap) layout permutations during shard/unshard.
"""

import numpy as np

import concourse.tile as tile
from concourse import bacc, mybir
from concourse.bass_utils import run_bass_kernel_spmd

# Problem constants (hardcoded per contract)
B_SZ, T, DIM = 32, 4096, 128
N_CORES = 8
B_PER = B_SZ // N_CORES      # 4 batch rows per core

PS_DT = mybir.dt.float32

_MM_DTS = {"f32r": mybir.dt.float32r, "bf16": mybir.dt.bfloat16,
           "fp16": mybir.dt.float16}

# On-chip matmul dtype. "f32r" (fp32 storage, ~tf32 matmul), "bf16", or
# "fp16" (half storage, 10-bit mantissa -- 8x less quantization error than
# bf16 at the same speed; all values here are << fp16 range).
# NOTE: a bf16 matmul followed by an f32r matmul hangs the PE (weight-path
# hazard, repro.py) -- so 2-byte and f32r modes must not be mixed.
DT_MODE = "fp16"
MM_DT = _MM_DTS[DT_MODE]
Y_HALF = True                # ship y in the 2-byte dtype (halves DMA-out)


def set_dtypes(mode, y_half=None):
    global DT_MODE, MM_DT, Y_HALF
    DT_MODE = mode
    MM_DT = _MM_DTS[mode]
    if y_half is not None:
        Y_HALF = y_half


def _np_dt():
    import ml_dtypes
    return {"f32r": np.float32, "bf16": ml_dtypes.bfloat16,
            "fp16": np.float16}[DT_MODE]

CW = 512                     # matmul column-chunk width (one PSUM bank of f32)
PSY_BUFS = 1
SCAN_BUFS = 2


def set_chunk(Lnew, Jnew):
    """Set chunk length L (scan steps) and boundary truncation J."""
    global L, N1, COLS, J, NW, W_A, W_B, W_F, W_M0, W_K0, NCH
    L = Lnew
    N1 = T // L
    COLS = N1 * B_PER
    J = Jnew
    NW = 3 + J + L
    W_A, W_B, W_F = 0, 1, 2
    W_M0 = 3
    W_K0 = 3 + J
    NCH = COLS // CW


set_chunk(16, 5)             # 0.9^(16*5) ~ 2.2e-4 truncation, << fp16 noise


def _build_program(loop_reps=1, variant="full"):
    nc = bacc.Bacc("TRN2", target_bir_lowering=False, debug=False,
                   num_devices=N_CORES)
    u_dt = MM_DT
    y_dt = MM_DT if (Y_HALF and DT_MODE != "f32r") else PS_DT
    ut = nc.dram_tensor("ut", [L, 128, COLS], u_dt, kind="ExternalInput")
    wt = nc.dram_tensor("wt", [128, NW * 128], MM_DT, kind="ExternalInput")
    if variant in ("dma", "dmaint", "dmain", "dmaout"):
        yt_dt = u_dt
    elif variant == "scan":
        yt_dt = MM_DT
    else:
        yt_dt = y_dt
    yt = nc.dram_tensor("yt", [L, 128, COLS], yt_dt, kind="ExternalOutput")

    with tile.TileContext(nc) as tc:
        from contextlib import ExitStack
        ctx = ExitStack()
        with (
            tc.tile_pool(name="wts", bufs=1) as wpool,
            tc.tile_pool(name="u", bufs=1) as upool,
            tc.tile_pool(name="x", bufs=1) as xpool,
            tc.tile_pool(name="s", bufs=1) as spool,
            tc.tile_pool(name="y", bufs=6) as ypool,
            tc.tile_pool(name="ps", bufs=SCAN_BUFS, space="PSUM") as pspool,
            tc.tile_pool(name="psy", bufs=PSY_BUFS, space="PSUM") as psypool,
            ctx,
        ):
            wtile = wpool.tile([128, NW, 128], MM_DT)
            nc.sync.dma_start(wtile[:], wt.ap().rearrange("p (n d) -> p n d",
                                                          n=NW))

            def w(i):
                return wtile[:, i, :]

            def wB():
                return w(W_B)

            def wF():
                return w(W_F)

            def cc(h):  # column-chunk slice
                return slice(h * CW, (h + 1) * CW)

            if loop_reps > 1:
                ctx.enter_context(tc.For_i(0, loop_reps, 1))

            n_loads = 1 if variant == "dmaout" else L
            u_tiles = []
            lag = 4
            for j in range(n_loads):
                u_j = upool.tile([128, COLS], u_dt, tag=f"u{j}", name=f"u{j}")
                nc.sync.dma_start(u_j[:], ut[j])
                u_tiles.append(u_j)
                if variant == "dmaint" and j >= lag:
                    nc.sync.dma_start(yt[j - lag], u_tiles[j - lag][:])
            if variant == "dmaint":
                for j in range(L - lag, L):
                    nc.sync.dma_start(yt[j], u_tiles[j][:])

            if variant == "dmaint":
                pass
            elif variant == "dmain":
                nc.sync.dma_start(yt[0], u_tiles[0][:])
            elif variant == "dma":
                for j in range(L):
                    nc.sync.dma_start(yt[j], u_tiles[j][:])
            elif variant == "dmaout":
                for j in range(L):
                    nc.sync.dma_start(yt[j], u_tiles[0][:])
            elif variant == "outs":
                for j in range(L):
                    for h in range(NCH):
                        ps_y = psypool.tile([128, CW], PS_DT, tag=f"y{h}",
                                            name=f"psy{j}_{h}")
                        nc.tensor.matmul(ps_y[:], wF(), u_tiles[j][:, cc(h)],
                                         start=True, stop=False)
                        nc.tensor.matmul(ps_y[:], w(W_K0 + j),
                                         u_tiles[j][:, cc(h)],
                                         start=False, stop=False)
                        nc.tensor.matmul(ps_y[:], w(W_K0),
                                         u_tiles[j][:, cc(h)],
                                         start=False, stop=True)
                        y_sb = ypool.tile([128, CW], y_dt, tag="ysb",
                                          name=f"y{j}_{h}")
                        if (j * NCH + h) % 2 == 0:
                            nc.scalar.copy(y_sb[:], ps_y[:])
                        else:
                            nc.vector.tensor_copy(y_sb[:], ps_y[:])
                        nc.sync.dma_start(yt[j, :, cc(h)], y_sb[:])
            else:
                # ---- Phase A: local scan; keep all w_{c,j} slabs ----
                x_tiles = [None]  # x_tiles[j] = W_j slab (j>=1); W_0 == 0
                for j in range(L):
                    x_j1 = xpool.tile([128, COLS], MM_DT, tag=f"x{j+1}",
                                      name=f"x{j+1}")
                    for h in range(NCH):
                        ps = pspool.tile([128, CW], PS_DT, tag=f"scan{h}",
                                         name=f"ps{j}_{h}")
                        nc.tensor.matmul(ps[:], wB(), u_tiles[j][:, cc(h)],
                                         start=True, stop=(j == 0))
                        if j > 0:
                            nc.tensor.matmul(ps[:], w(W_A),
                                             x_tiles[j][:, cc(h)],
                                             start=False, stop=True)
                        if h % 2 == 0:
                            nc.scalar.copy(x_j1[:, cc(h)], ps[:])
                        else:
                            nc.vector.tensor_copy(x_j1[:, cc(h)], ps[:])
                    x_tiles.append(x_j1)

                if variant == "scan":
                    nc.sync.dma_start(yt[0], x_tiles[L][:])
                else:
                    # ---- Phase B: chunk-start states (truncated conv) ----
                    # s_sb[:, 0:B_PER] is never read (chunk 0 has s=0).
                    r_tile = x_tiles[L]
                    s_tiles = []
                    for h in range(NCH):
                        s_sb_h = spool.tile([128, CW], MM_DT, tag=f"s{h}",
                                            name=f"s_sb{h}")
                        s_tiles.append(s_sb_h)
                    for h in range(NCH):
                        ps_s = psypool.tile([128, CW], PS_DT, tag=f"s{h}",
                                            bufs=1, name=f"ps_s{h}")
                        lo = h * CW          # output col range [lo, hi)
                        for m in range(J):
                            sh = (m + 1) * B_PER
                            olo = max(lo, sh)
                            ncols = CW - (olo - lo)
                            nc.tensor.matmul(
                                ps_s[:, olo - lo:CW], w(W_M0 + m),
                                r_tile[:, olo - sh:olo - sh + ncols],
                                start=(m == 0), stop=(m == J - 1))
                        olo = 0 if h > 0 else B_PER
                        if h % 2 == 0:
                            nc.scalar.copy(s_tiles[h][:, olo:CW],
                                           ps_s[:, olo:CW])
                        else:
                            nc.vector.tensor_copy(s_tiles[h][:, olo:CW],
                                                  ps_s[:, olo:CW])

                    # ---- Phase C: outputs ----
                    for j in range(L):
                        y_sb = ypool.tile([128, COLS], y_dt, tag="ysb",
                                          name=f"y{j}")
                        for h in range(NCH):
                            ps_y = psypool.tile([128, CW], PS_DT, tag=f"y{h}",
                                                name=f"psy{j}_{h}")
                            nc.tensor.matmul(ps_y[:], wF(),
                                             u_tiles[j][:, cc(h)],
                                             start=True, stop=False)
                            slo = 0 if h > 0 else B_PER
                            nc.tensor.matmul(ps_y[:, slo:CW],
                                             w(W_K0 + j), s_tiles[h][:, slo:CW],
                                             start=False, stop=(j == 0))
                            if j > 0:
                                nc.tensor.matmul(ps_y[:], w(W_K0),
                                                 x_tiles[j][:, cc(h)],
                                                 start=False, stop=True)
                            if h % 2 == 0:
                                nc.scalar.copy(y_sb[:, cc(h)], ps_y[:])
                            else:
                                nc.vector.tensor_copy(y_sb[:, cc(h)], ps_y[:])
                        nc.sync.dma_start(yt[j], y_sb[:])

    nc.compile()
    return nc


_cached_nc = None


def _get_program():
    global _cached_nc
    if _cached_nc is None:
        _cached_nc = _build_program()
    return _cached_nc


def _make_weights(A, B, C, D):
    A = np.asarray(A, np.float64)
    Bm = np.asarray(B, np.float64)
    C = np.asarray(C, np.float64)
    Dm = np.asarray(D, np.float64)
    E = C @ A
    F = C @ Bm + Dm
    wts = np.zeros((NW, 128, 128), np.float64)
    wts[W_A] = A.T
    wts[W_B] = Bm.T
    wts[W_F] = F.T
    AL = np.linalg.matrix_power(A, L)
    Mm = np.eye(128)
    for m in range(J):
        wts[W_M0 + m] = Mm.T
        Mm = Mm @ AL
    Aj = np.eye(128)
    for j in range(L):
        wts[W_K0 + j] = (E @ Aj).T
        Aj = Aj @ A
    # ship pre-transposed [128, NW*128] so the SBUF load is contiguous
    wts_t = wts.transpose(1, 0, 2).reshape(128, NW * 128)
    return np.ascontiguousarray(wts_t.astype(_np_dt()))


def make_in_maps(u, A, B, C, D):
    u = np.asarray(u, np.float32)
    wts = _make_weights(A, B, C, D)
    np_dt = _np_dt()
    in_maps = []
    for core in range(N_CORES):
        uc = u[core * B_PER:(core + 1) * B_PER]            # [4, T, 128]
        # ut[j, d, c*B_PER + b] = uc[b, c*L + j, d]
        ut = uc.reshape(B_PER, N1, L, DIM).transpose(2, 3, 1, 0)
        ut = np.ascontiguousarray(ut).reshape(L, 128, COLS).astype(np_dt)
        in_maps.append({"ut": ut, "wt": wts})
    return in_maps


def kernel(inputs, A, B, C, D):
    nc = _get_program()
    in_maps = make_in_maps(inputs, A, B, C, D)

    res = run_bass_kernel_spmd(nc, in_maps, core_ids=list(range(N_CORES)))

    out = np.empty((B_SZ, T, DIM), np.float32)
    for core in range(N_CORES):
        ytc = np.asarray(res.results[core]["yt"], np.float32)  # [L, 128, COLS]
        # out[b, c*L + j, d] = ytc[j, d, c*B_PER + b]
        oc = ytc.reshape(L, DIM, N1, B_PER).transpose(3, 2, 0, 1)
        out[core * B_PER:(core + 1) * B_PER] = oc.reshape(B_PER, T, DIM)
    return out

